# revision 22
# baseline (speedup 1.0000x reference)
"""BERT self-attention layer (B=8, S=1024, H=12, Dh=64) on 8 trn2 NeuronCores.

Sharding: pure data-parallel over batch (1 batch item per core, weights
replicated).

Matmul path runs in fp8e4m3 with DoubleRow perf mode where the contraction
is >=256 (QKV projections 6->3 passes, ctx 8->4, dense 6->3).  Scores stay
single-pass fp8 (K=64).  The residual + LN path stays exact fp32; the
residual dominates the output (dense branch is ~1% of it), so fp8 noise in
the attention path dilutes ~100x and the final error is ~1e-4 relative.

Layouts (T = features on partitions):
  x_all  [128, 6*1024] fp8  xT, col = kt*1024 + q        (DR pairs via view)
  w*_all [128, 6*768]  fp8  W^T, col = kt*768 + fo
  qT/kT  6 x [128, 1024] fp8 per head-pair
  vpair  [128, 4*1560] fp8  V natural + ones col: col = jp*1560 + (j%2)*780
                            + 65*h + c   (c=64 is the ones/denominator col)
  e      [128, 2*1024] fp8 per (half, jp): exp(scores) pairs for DR ctx
  ctx_all[128, 6*1024] fp8  normalized ctx^T, col = kt*1024 + q

Per-core dataflow:
  xT   = PE-transpose(x), evac cast on gpsimd
  w^T  = 32x32-permuted DMA + DVE StreamTranspose + cast to fp8
  QT/KT= DR(w^T, xT) per head-pair, DVE evac
  V    = DR(xT, wv^T) natural layout, DVE strided evac into vpair
  per head pair, per j: sT = K^T-slice @ QT (fp8, psum), e = exp(sT/8+mask)
  ctx  = DR(vpair, e-pairs) accumulated over 4 jp -> [65, S]; row 64 = denom
  ctxT = cc[0:64] * bcast(1/cc[64])  (DVE mul, gpsimd bcast) -> fp8
  out  = LN(x + DR(ctxT, wd^T))  fused via STT/accum_out
"""

import os
import numpy as np
from contextlib import ExitStack

import concourse.bass as bass
import concourse.bacc as bacc
import concourse.tile as tile
from concourse import mybir
from concourse._compat import with_exitstack
from concourse.bass import ts, ds
from concourse.bass_utils import run_bass_kernel_spmd
from concourse.masks import make_identity

H = 12
DH = 64
D = 768
S = 1024
P = 128
KT_ = D // P  # 6 feature tiles
ST_ = S // P  # 8 sequence tiles
HB = 128  # per-head V block width: 64 value cols + ones col at 64 + 63
          # garbage pad cols (DoubleRow ldweights requires stationary width
          # of exactly 32/64/128; psum rows 65-127 are never read)
VW = H * HB  # 1536
EPS = 1e-12
F32 = mybir.dt.float32
FP8 = mybir.dt.float8e4  # e4m3
U8 = mybir.dt.uint8
FT = mybir.ActivationFunctionType
ALU = mybir.AluOpType
DR = mybir.MatmulPerfMode.DoubleRow
N_CORES = 8
ONE_FP8 = 0x38  # fp8e4m3 encoding of 1.0


def _permuted_src(ap, col0, n_free_blocks):
    """DRAM AP enumerating src[32J+r, col0+c] for r,c in 32x32 blocks, in
    (r, J, c) order — the 32x32-block-permuted load feeding StreamTranspose."""
    rs = ap.ap[0][0]
    return bass.AP(
        tensor=ap.tensor,
        offset=ap.offset + col0,
        ap=[[rs, 32], [32 * rs, n_free_blocks], [1, 32]],
    )


def _load_wT(nc, dest, scratch, src_ap, dma_engines, cast_engine, tag):
    """Transpose a [768, 768] DRAM weight into dest (a [128, 6*768] fp8
    tile, col = kt*768 + fo) without touching the PE: permuted DMA ->
    DVE 32x32 StreamTranspose -> cast to fp8."""
    for kt in range(KT_):
        perm = scratch.tile([P, D], F32, tag="tsp", bufs=3, name=f"{tag}p")
        p4 = perm.rearrange("(i r) (j c) -> i r j c", r=32, c=32)
        for i in range(4):
            dma_engines[(kt * 4 + i) % len(dma_engines)].dma_start(
                out=p4[i],
                in_=_permuted_src(src_ap, 128 * kt + 32 * i, D // 32),
            )
        tf = scratch.tile([P, D], F32, tag="tst", bufs=3, name=f"{tag}t")
        nc.vector.transpose(tf, perm)
        cast_engine.tensor_copy(dest[:, ds(kt * D, D)], tf)


def _bcast_load(nc, out_tile, vec_ap, n_part):
    """DMA a [N] DRAM vector replicated across n_part partitions."""
    src = bass.AP(
        tensor=vec_ap.tensor,
        offset=vec_ap.offset,
        ap=[[0, n_part]] + [list(d) for d in vec_ap.ap],
    )
    nc.gpsimd.dma_start(out=out_tile, in_=src)


@with_exitstack
def bert_attn_kernel(
    ctx: ExitStack,
    tc: tile.TileContext,
    out_ap: bass.AP,
    x_ap: bass.AP,
    mask_ap: bass.AP,
    wq_ap: bass.AP,
    bq_ap: bass.AP,
    wk_ap: bass.AP,
    bk_ap: bass.AP,
    wv_ap: bass.AP,
    bv_ap: bass.AP,
    wd_ap: bass.AP,
    bd_ap: bass.AP,
    g_ap: bass.AP,
    b_ap: bass.AP,
    use_mask: bool,
    use_qkv_bias: bool,
    use_dense_bias: bool,
    use_ln_affine: bool,
):
    nc = tc.nc

    # ---- persistent pools ----
    const_pool = ctx.enter_context(tc.tile_pool(name="const", bufs=1))
    big_pool = ctx.enter_context(tc.tile_pool(name="big", bufs=1))

    eps_t = const_pool.tile([P, 1], F32)
    nc.vector.memset(eps_t, EPS)
    ident = const_pool.tile([P, P], F32)
    make_identity(nc, ident)

    maskT = None
    if use_mask:
        maskT = const_pool.tile([P, ST_], F32)
        nc.sync.dma_start(out=maskT, in_=mask_ap.rearrange("(t p) -> p t", p=P))

    bq_t = bk_t = bv_bc = None
    if use_qkv_bias:
        bq_t = const_pool.tile([P, KT_], F32)
        nc.sync.dma_start(out=bq_t, in_=bq_ap.rearrange("(t p) -> p t", p=P))
        bk_t = const_pool.tile([P, KT_], F32)
        nc.sync.dma_start(out=bk_t, in_=bk_ap.rearrange("(t p) -> p t", p=P))
        bv_bc = const_pool.tile([P, D], F32)
        _bcast_load(nc, bv_bc, bv_ap, P)
    bd_bc = None
    if use_dense_bias:
        bd_bc = const_pool.tile([P, D], F32)
        _bcast_load(nc, bd_bc, bd_ap, P)
    g_bc = b_bc = None
    if use_ln_affine:
        g_bc = const_pool.tile([P, D], F32)
        _bcast_load(nc, g_bc, g_ap, P)
        b_bc = const_pool.tile([P, D], F32)
        _bcast_load(nc, b_bc, b_ap, P)

    # persistent data tiles
    x_all = big_pool.tile([P, KT_ * S], FP8, name="x_all")
    xv = x_all.rearrange("p (k q) -> p k q", q=S)
    wq_all = big_pool.tile([P, KT_ * D], FP8, name="wq_all")
    wk_all = big_pool.tile([P, KT_ * D], FP8, name="wk_all")
    wv_all = big_pool.tile([P, KT_ * D], FP8, name="wv_all")
    qT = [big_pool.tile([P, S], FP8, name=f"qT{i}") for i in range(KT_)]
    kT = [big_pool.tile([P, S], FP8, name=f"kT{i}") for i in range(KT_)]
    vpair = big_pool.tile([P, 4 * 2 * VW], FP8, name="vpair")
    vv = vpair.rearrange("p (j t h c) -> p j t h c", t=2, h=H, c=HB)
    xn = [big_pool.tile([P, D], F32, tag="xn", bufs=ST_, name=f"xn{i}")
          for i in range(ST_)]
    ctx_all = big_pool.tile([P, KT_ * S], FP8, name="ctx_all")
    cxv = ctx_all.rearrange("p (k q) -> p k q", q=S)
    wd_all = big_pool.tile([P, KT_ * D], FP8, name="wd_all")

    # Head-block column map: 0:64 = values, 64 = ones (denominator row),
    # 65:127 = filler ones feeding unread psum rows (deterministic, no
    # uninitialized weights entering the PE).
    for jp in range(4):
        nc.gpsimd.memset(vv[:, jp, :, :, DH:HB].bitcast(U8), ONE_FP8)

    # =========== phase 1: x + weight loads (row-major, 4 queues) ===========
    # x: row-major load; all transposes happen on the PE (idle here anyway);
    # the DVE is kept free for evac casts so the attention pipe fills fast.
    for st in range(ST_):
        nc.sync.dma_start(out=xn[st], in_=x_ap[ts(st, P), :])

    wsc_pool = ctx.enter_context(tc.tile_pool(name="wsc", bufs=1))
    wnat = {}
    for wtag, w_ap, dma_eng in (("wq", wq_ap, nc.scalar),
                                ("wk", wk_ap, nc.scalar),
                                ("wv", wv_ap, nc.gpsimd)):
        tiles = [wsc_pool.tile([P, D], F32, tag="wnat", bufs=3 * KT_,
                               name=f"{wtag}n{r}") for r in range(KT_)]
        for r in range(KT_):
            dma_eng.dma_start(out=tiles[r], in_=w_ap[ts(r, P), :])
        wnat[wtag] = tiles

    # =========== phase 2: transposes + QKV projections (DoubleRow) ========
    wqv = wq_all.rearrange("p (k f) -> p k f", f=D)
    wkv = wk_all.rearrange("p (k f) -> p k f", f=D)
    wvv = wv_all.rearrange("p (k f) -> p k f", f=D)

    with tc.tile_pool(name="ps_tv", bufs=2, space="PSUM") as psum_tv, \
         tc.tile_pool(name="ps_qk", bufs=2, space="PSUM") as psum_qk:

        def transpose_in(src_tiles, dest_all, evac):
            # block-transpose 6x6 [128,128] tiles; evac one [128,768]
            # feature-tile column at a time
            for c in range(KT_):
                tps = psum_tv.tile([P, D], F32, tag="tps", bufs=2,
                                   name="tps")
                t3 = tps.rearrange("p (k b) -> p k b", b=P)
                for r in range(KT_):
                    nc.tensor.transpose(t3[:, r], src_tiles[r][:, ts(c, P)],
                                        ident)
                evac(c, tps, t3)

        # xT: evac on ScalarE (strided into x_all), DVE stays free
        for st in range(ST_):
            tps = psum_tv.tile([P, D], F32, tag="tps", bufs=2, name="tpsx")
            t3 = tps.rearrange("p (k b) -> p k b", b=P)
            for kt in range(KT_):
                nc.tensor.transpose(t3[:, kt], xn[st][:, ts(kt, P)], ident)
            nc.scalar.copy(xv[:, :, ds(st * P, P)], t3)

        transpose_in(wnat["wq"], wq_all,
                     lambda c, tps, t3: nc.vector.tensor_copy(
                         wq_all[:, ds(c * D, D)], tps))
        transpose_in(wnat["wk"], wk_all,
                     lambda c, tps, t3: nc.vector.tensor_copy(
                         wk_all[:, ds(c * D, D)], tps))

        def qk_proj(pr):
            for wv3, bias_t, dest in ((wqv, bq_t, qT), (wkv, bk_t, kT)):
                qps = psum_qk.tile([P, S], F32, tag="qkps", bufs=2,
                                   name="qps")
                for p2 in range(KT_ // 2):
                    for qc in range(0, S, 512):
                        nc.tensor.matmul(
                            qps[:, ds(qc, 512)],
                            lhsT=wv3[:, 2 * p2 : 2 * p2 + 2, ts(pr, P)],
                            rhs=xv[:, 2 * p2 : 2 * p2 + 2, ds(qc, 512)],
                            start=(p2 == 0),
                            stop=(p2 == KT_ // 2 - 1),
                            perf_mode=DR,
                        )
                if use_qkv_bias:
                    nc.vector.tensor_scalar_add(dest[pr], qps,
                                                bias_t[:, pr : pr + 1])
                else:
                    nc.vector.tensor_copy(dest[pr], qps)

        qk_proj(0)
        transpose_in(wnat["wv"], wv_all,
                     lambda c, tps, t3: nc.vector.tensor_copy(
                         wv_all[:, ds(c * D, D)], tps))
        qk_proj(1)

        # V natural layout, per-head blocks in vpair
        for st in range(ST_):
            vps = psum_tv.tile([P, D], F32, tag="tps", bufs=2, name="vps")
            for p2 in range(KT_ // 2):
                for c0, cw in ((0, 512), (512, 256)):
                    nc.tensor.matmul(
                        vps[:, ds(c0, cw)],
                        lhsT=xv[:, 2 * p2 : 2 * p2 + 2, ts(st, P)],
                        rhs=wvv[:, 2 * p2 : 2 * p2 + 2, ds(c0, cw)],
                        start=(p2 == 0),
                        stop=(p2 == KT_ // 2 - 1),
                        perf_mode=DR,
                    )
            v3 = vps.rearrange("p (h c) -> p h c", c=DH)
            vdst = vv[:, st // 2, st % 2, :, 0:DH]
            if use_qkv_bias:
                stage = psum_tv.tile([P, D], F32, tag="tps", bufs=2,
                                     name="vstage")
                s3 = stage.rearrange("p (h c) -> p h c", c=DH)
                bv3 = bv_bc.rearrange("p (h c) -> p h c", c=DH)
                nc.vector.tensor_add(s3, v3, bv3)
                nc.vector.tensor_copy(vdst, s3)
            else:
                nc.vector.tensor_copy(vdst, v3)

        for pr in range(2, KT_):
            qk_proj(pr)

    # =========== phase 3: attention, two heads at a time ===========
    wdv = wd_all.rearrange("p (k f) -> p k f", f=D)
    with tc.tile_pool(name="expT", bufs=1) as exp_pool, \
         tc.tile_pool(name="den", bufs=1) as den_pool, \
         tc.tile_pool(name="ps_s", bufs=2, space="PSUM") as psum_s, \
         tc.tile_pool(name="ps_ctx", bufs=2, space="PSUM") as psum_ctx:

        def emit_ctx(pend):
            # ctx DoubleRow matmuls for one deferred (pair, jp) group
            pr, jp, cc, ets = pend
            for half in range(2):
                h = 2 * pr + half
                e3 = ets[half].rearrange("p (t q) -> p t q", q=S)
                for qc in range(0, S, 512):
                    nc.tensor.matmul(
                        cc[half][:, ds(qc, 512)],
                        lhsT=vv[:, jp, :, h, :],
                        rhs=e3[:, :, ds(qc, 512)],
                        start=(jp == 0),
                        stop=(jp == 3),
                        perf_mode=DR,
                    )
            if jp == 3:
                emit_den(pr, cc)

        def emit_den(pr, cc):
            # normalize: ctxT = cc[0:64] / cc[64] into ctx_all (fp8)
            for half in range(2):
                h = 2 * pr + half
                kt = h // 2
                den_sb = den_pool.tile([1, S], F32, tag="den_sb", bufs=2)
                nc.vector.tensor_copy(den_sb, cc[half][DH : DH + 1, :])
                rec = den_pool.tile([1, S], F32, tag="rec", bufs=2)
                nc.vector.reciprocal_approx_fast(rec, den_sb)
                recb = den_pool.tile([DH, S], F32, tag="recb", bufs=2)
                nc.gpsimd.partition_broadcast(recb, rec)
                nc.vector.tensor_mul(
                    ctx_all[DH * (h % 2) : DH * (h % 2) + DH, ts(kt, S)],
                    cc[half][0:DH, :], recb)

        pending = None  # deferred ctx group: hides under the next jp's exps
        for pr in range(H // 2):
            if pr == 1:
                # overlap the Wd transpose-load with attention compute
                _load_wT(nc, wd_all, wsc_pool, wd_ap, [nc.gpsimd, nc.sync],
                         nc.vector, "wd")
            cc = [psum_ctx.tile([HB, S], F32, tag="cps", bufs=2,
                                name=f"cps{half}") for half in range(2)]
            et = [None, None]
            for j in range(ST_):
                jp, jh = j // 2, j % 2
                for half in range(2):
                    hp = DH * half
                    if jh == 0:
                        et[half] = exp_pool.tile([P, 2 * S], FP8,
                                                 tag=f"e{half}", bufs=2,
                                                 name=f"e{half}")
                    sps = psum_s.tile([P, S], F32, tag="sps", bufs=2,
                                      name=f"sps{half}")
                    for qc in range(0, S, 512):
                        nc.tensor.matmul(
                            sps[:, ds(qc, 512)],
                            lhsT=kT[pr][hp : hp + DH, ts(j, P)],
                            rhs=qT[pr][hp : hp + DH, ds(qc, 512)],
                            start=True,
                            stop=True,
                        )
                    nc.scalar.activation(
                        et[half][:, ds(jh * S, S)], sps, FT.Exp,
                        bias=(maskT[:, j : j + 1] if use_mask else 0.0),
                        scale=0.125,
                    )
                if jh == 1:
                    if pending is not None:
                        emit_ctx(pending)
                    pending = (pr, jp, cc, (et[0], et[1]))
        emit_ctx(pending)

    # =========== phase 4: dense + residual + layernorm ===========
    with tc.tile_pool(name="ln", bufs=2) as ln_pool, \
         tc.tile_pool(name="stat", bufs=4) as stat_pool, \
         tc.tile_pool(name="osb", bufs=3) as out_pool, \
         tc.tile_pool(name="ps_o", bufs=2, space="PSUM") as psum_o:

        for st in range(ST_):
            xr = xn[st]
            if use_dense_bias:
                xb = ln_pool.tile([P, D], F32, tag="xb")
                nc.vector.tensor_add(xb, xr, bd_bc)
                xr = xb
            ops = psum_o.tile([P, D], F32, tag="ops", bufs=2)
            for p2 in range(KT_ // 2):
                for c0, cw in ((0, 512), (512, 256)):
                    nc.tensor.matmul(
                        ops[:, ds(c0, cw)],
                        lhsT=cxv[:, 2 * p2 : 2 * p2 + 2, ts(st, P)],
                        rhs=wdv[:, 2 * p2 : 2 * p2 + 2, ds(c0, cw)],
                        start=(p2 == 0),
                        stop=(p2 == KT_ // 2 - 1),
                        perf_mode=DR,
                    )
            # full = dense_out + x, accumulating the row-sum on the fly
            full = ln_pool.tile([P, D], F32, tag="full")
            sums = stat_pool.tile([P, 1], F32, tag="sums")
            nc.vector.scalar_tensor_tensor(
                out=full, in0=ops, scalar=1.0, in1=xr,
                op0=ALU.mult, op1=ALU.add, accum_out=sums,
            )
            # sum of squares on ScalarE; sq is a dead store
            sq = ln_pool.tile([P, D], F32, tag="sq")
            ssq = stat_pool.tile([P, 1], F32, tag="ssq")
            nc.scalar.activation(sq, full, FT.Square, accum_out=ssq)
            mu = stat_pool.tile([P, 1], F32, tag="mu")
            nc.vector.tensor_scalar_mul(mu, sums, 1.0 / D)
            mu2 = stat_pool.tile([P, 1], F32, tag="mu2")
            nc.vector.tensor_scalar_mul(mu2, mu, mu)
            var = stat_pool.tile([P, 1], F32, tag="var")
            nc.vector.scalar_tensor_tensor(
                out=var, in0=ssq, scalar=1.0 / D, in1=mu2,
                op0=ALU.mult, op1=ALU.subtract,
            )
            std = stat_pool.tile([P, 1], F32, tag="std")
            nc.scalar.activation(std, var, FT.Sqrt, bias=eps_t)
            rstd = stat_pool.tile([P, 1], F32, tag="rstd")
            nc.vector.reciprocal(rstd, std)
            osb = out_pool.tile([P, D], F32, tag="osb")
            nc.vector.tensor_scalar(
                out=osb, in0=full, scalar1=mu, scalar2=rstd,
                op0=ALU.subtract, op1=ALU.mult,
            )
            if use_ln_affine:
                nc.vector.tensor_mul(osb, osb, g_bc)
                nc.vector.tensor_add(osb, osb, b_bc)
            nc.sync.dma_start(out=out_ap[ts(st, P), :], in_=osb)


def build(flags):
    nc = bacc.Bacc(
        "TRN2", target_bir_lowering=False, debug=False, num_devices=N_CORES
    )
    aps = {}
    for name, shape in (
        ("hidden_states", [S, D]),
        ("attention_mask", [S]),
        ("Wq", [D, D]), ("bq", [D]),
        ("Wk", [D, D]), ("bk", [D]),
        ("Wv", [D, D]), ("bv", [D]),
        ("Wd", [D, D]), ("bd", [D]),
        ("ln_g", [D]), ("ln_b", [D]),
    ):
        aps[name] = nc.dram_tensor(name, shape, F32, kind="ExternalInput").ap()
    out = nc.dram_tensor("out", [S, D], F32, kind="ExternalOutput").ap()

    with tile.TileContext(nc) as tc:
        bert_attn_kernel(
            tc, out,
            aps["hidden_states"], aps["attention_mask"],
            aps["Wq"], aps["bq"], aps["Wk"], aps["bk"],
            aps["Wv"], aps["bv"], aps["Wd"], aps["bd"],
            aps["ln_g"], aps["ln_b"],
            *flags,
        )
    nc.compile()
    return nc


_CACHE = {}
last_results = None  # BassKernelResults of the most recent run (for test.py)


def kernel(**inputs):
    xs = {k: np.ascontiguousarray(np.asarray(v, dtype=np.float32))
          for k, v in inputs.items()}
    B = xs["hidden_states"].shape[0]
    assert B == N_CORES

    flags = (
        bool(np.any(xs["attention_mask"])),
        bool(np.any(xs["bq"]) or np.any(xs["bk"]) or np.any(xs["bv"])),
        bool(np.any(xs["bd"])),
        bool(np.any(xs["ln_g"] != 1.0) or np.any(xs["ln_b"])),
    )
    if flags not in _CACHE:
        _CACHE[flags] = build(flags)
    nc = _CACHE[flags]

    shared = {k: xs[k] for k in
              ("Wq", "bq", "Wk", "bk", "Wv", "bv", "Wd", "bd", "ln_g", "ln_b")}
    in_maps = [
        dict(
            hidden_states=xs["hidden_states"][i],
            attention_mask=np.ascontiguousarray(
                xs["attention_mask"][i].reshape(S)),
            **shared,
        )
        for i in range(N_CORES)
    ]
    trace = bool(int(os.environ.get("BERT_KERNEL_TRACE", "0")))
    res = run_bass_kernel_spmd(
        nc, in_maps, core_ids=list(range(N_CORES)), trace=trace
    )
    global last_results
    last_results = res
    return np.stack([res.results[i]["out"] for i in range(N_CORES)], axis=0)


if __name__ == "__main__":
    rng = np.random.default_rng(0)
    ins = {
        "hidden_states": rng.standard_normal((8, S, D), dtype=np.float32),
        "attention_mask": np.zeros((8, 1, 1, S), np.float32),
        "Wq": rng.standard_normal((D, D), dtype=np.float32) * 0.02,
        "bq": np.zeros(D, np.float32),
        "Wk": rng.standard_normal((D, D), dtype=np.float32) * 0.02,
        "bk": np.zeros(D, np.float32),
        "Wv": rng.standard_normal((D, D), dtype=np.float32) * 0.02,
        "bv": np.zeros(D, np.float32),
        "Wd": rng.standard_normal((D, D), dtype=np.float32) * 0.02,
        "bd": np.zeros(D, np.float32),
        "ln_g": np.ones(D, np.float32),
        "ln_b": np.zeros(D, np.float32),
    }
    out = kernel(**ins)
    print(out.shape, out.dtype, np.abs(out).max())


# revision 24
# speedup vs baseline: 1.1266x; 1.1266x over previous
"""BERT self-attention layer (B=8, S=1024, H=12, Dh=64) on 8 trn2 NeuronCores.

Sharding: pure data-parallel over batch (1 batch item per core, weights
replicated).

Matmul path runs in fp8e4m3 with DoubleRow perf mode where the contraction
is >=256 (QKV projections 6->3 passes, ctx 8->4, dense 6->3).  Scores stay
single-pass fp8 (K=64).  The residual + LN path stays exact fp32; the
residual dominates the output (dense branch is ~1% of it), so fp8 noise in
the attention path dilutes ~100x and the final error is ~1e-4 relative.

Layouts (T = features on partitions):
  x_all  [128, 6*1024] fp8  xT, col = kt*1024 + q        (DR pairs via view)
  w*_all [128, 6*768]  fp8  W^T, col = kt*768 + fo
  qT/kT  6 x [128, 1024] fp8 per head-pair
  vpair  [128, 4*1560] fp8  V natural + ones col: col = jp*1560 + (j%2)*780
                            + 65*h + c   (c=64 is the ones/denominator col)
  e      [128, 2*1024] fp8 per (half, jp): exp(scores) pairs for DR ctx
  ctx_all[128, 6*1024] fp8  normalized ctx^T, col = kt*1024 + q

Per-core dataflow:
  xT   = PE-transpose(x), evac cast on gpsimd
  w^T  = 32x32-permuted DMA + DVE StreamTranspose + cast to fp8
  QT/KT= DR(w^T, xT) per head-pair, DVE evac
  V    = DR(xT, wv^T) natural layout, DVE strided evac into vpair
  per head pair, per j: sT = K^T-slice @ QT (fp8, psum), e = exp(sT/8+mask)
  ctx  = DR(vpair, e-pairs) accumulated over 4 jp -> [65, S]; row 64 = denom
  ctxT = cc[0:64] * bcast(1/cc[64])  (DVE mul, gpsimd bcast) -> fp8
  out  = LN(x + DR(ctxT, wd^T))  fused via STT/accum_out
"""

import os
import numpy as np
from contextlib import ExitStack

import concourse.bass as bass
import concourse.bacc as bacc
import concourse.tile as tile
from concourse import mybir
from concourse._compat import with_exitstack
from concourse.bass import ts, ds
from concourse.bass_utils import run_bass_kernel_spmd
from concourse.masks import make_identity

H = 12
DH = 64
D = 768
S = 1024
P = 128
KT_ = D // P  # 6 feature tiles
ST_ = S // P  # 8 sequence tiles
HB = 128  # per-head V block width: 64 value cols + ones col at 64 + 63
          # garbage pad cols (DoubleRow ldweights requires stationary width
          # of exactly 32/64/128; psum rows 65-127 are never read)
VW = H * HB  # 1536
EPS = 1e-12
F32 = mybir.dt.float32
FP8 = mybir.dt.float8e4  # e4m3
U8 = mybir.dt.uint8
FT = mybir.ActivationFunctionType
ALU = mybir.AluOpType
DR = mybir.MatmulPerfMode.DoubleRow
N_CORES = 8
ONE_FP8 = 0x38  # fp8e4m3 encoding of 1.0


def _permuted_src(ap, col0, n_free_blocks):
    """DRAM AP enumerating src[32J+r, col0+c] for r,c in 32x32 blocks, in
    (r, J, c) order — the 32x32-block-permuted load feeding StreamTranspose."""
    rs = ap.ap[0][0]
    return bass.AP(
        tensor=ap.tensor,
        offset=ap.offset + col0,
        ap=[[rs, 32], [32 * rs, n_free_blocks], [1, 32]],
    )


def _load_wT(nc, dest, scratch, src_ap, dma_engines, cast_engine, tag):
    """Transpose a [768, 768] DRAM weight into dest (a [128, 6*768] fp8
    tile, col = kt*768 + fo) without touching the PE: permuted DMA ->
    DVE 32x32 StreamTranspose -> cast to fp8."""
    for kt in range(KT_):
        perm = scratch.tile([P, D], F32, tag="tsp", bufs=3, name=f"{tag}p")
        p4 = perm.rearrange("(i r) (j c) -> i r j c", r=32, c=32)
        for i in range(4):
            dma_engines[(kt * 4 + i) % len(dma_engines)].dma_start(
                out=p4[i],
                in_=_permuted_src(src_ap, 128 * kt + 32 * i, D // 32),
            )
        tf = scratch.tile([P, D], F32, tag="tst", bufs=3, name=f"{tag}t")
        nc.vector.transpose(tf, perm)
        cast_engine.tensor_copy(dest[:, ds(kt * D, D)], tf)


def _bcast_load(nc, out_tile, vec_ap, n_part):
    """DMA a [N] DRAM vector replicated across n_part partitions."""
    src = bass.AP(
        tensor=vec_ap.tensor,
        offset=vec_ap.offset,
        ap=[[0, n_part]] + [list(d) for d in vec_ap.ap],
    )
    nc.gpsimd.dma_start(out=out_tile, in_=src)


@with_exitstack
def bert_attn_kernel(
    ctx: ExitStack,
    tc: tile.TileContext,
    out_ap: bass.AP,
    x_ap: bass.AP,
    mask_ap: bass.AP,
    wq_ap: bass.AP,
    bq_ap: bass.AP,
    wk_ap: bass.AP,
    bk_ap: bass.AP,
    wv_ap: bass.AP,
    bv_ap: bass.AP,
    wd_ap: bass.AP,
    bd_ap: bass.AP,
    g_ap: bass.AP,
    b_ap: bass.AP,
    use_mask: bool,
    use_qkv_bias: bool,
    use_dense_bias: bool,
    use_ln_affine: bool,
):
    nc = tc.nc

    # ---- persistent pools ----
    const_pool = ctx.enter_context(tc.tile_pool(name="const", bufs=1))
    big_pool = ctx.enter_context(tc.tile_pool(name="big", bufs=1))

    eps_t = const_pool.tile([P, 1], F32)
    nc.vector.memset(eps_t, EPS)
    ident = const_pool.tile([P, P], F32)
    make_identity(nc, ident)

    maskT = None
    if use_mask:
        maskT = const_pool.tile([P, ST_], F32)
        nc.sync.dma_start(out=maskT, in_=mask_ap.rearrange("(t p) -> p t", p=P))

    bq_t = bk_t = bv_bc = None
    if use_qkv_bias:
        bq_t = const_pool.tile([P, KT_], F32)
        nc.sync.dma_start(out=bq_t, in_=bq_ap.rearrange("(t p) -> p t", p=P))
        bk_t = const_pool.tile([P, KT_], F32)
        nc.sync.dma_start(out=bk_t, in_=bk_ap.rearrange("(t p) -> p t", p=P))
        bv_bc = const_pool.tile([P, D], F32)
        _bcast_load(nc, bv_bc, bv_ap, P)
    bd_bc = None
    if use_dense_bias:
        bd_bc = const_pool.tile([P, D], F32)
        _bcast_load(nc, bd_bc, bd_ap, P)
    g_bc = b_bc = None
    if use_ln_affine:
        g_bc = const_pool.tile([P, D], F32)
        _bcast_load(nc, g_bc, g_ap, P)
        b_bc = const_pool.tile([P, D], F32)
        _bcast_load(nc, b_bc, b_ap, P)

    # persistent data tiles
    x_all = big_pool.tile([P, KT_ * S], FP8, name="x_all")
    xv = x_all.rearrange("p (k q) -> p k q", q=S)
    wq_all = big_pool.tile([P, KT_ * D], FP8, name="wq_all")
    wk_all = big_pool.tile([P, KT_ * D], FP8, name="wk_all")
    wv_all = big_pool.tile([P, KT_ * D], FP8, name="wv_all")
    qT = [big_pool.tile([P, S], FP8, name=f"qT{i}") for i in range(KT_)]
    kT = [big_pool.tile([P, S], FP8, name=f"kT{i}") for i in range(KT_)]
    vpair = big_pool.tile([P, 4 * 2 * VW], FP8, name="vpair")
    vv = vpair.rearrange("p (j t h c) -> p j t h c", t=2, h=H, c=HB)
    xn = [big_pool.tile([P, D], F32, tag="xn", bufs=ST_, name=f"xn{i}")
          for i in range(ST_)]
    ctx_all = big_pool.tile([P, KT_ * S], FP8, name="ctx_all")
    cxv = ctx_all.rearrange("p (k q) -> p k q", q=S)
    wd_all = big_pool.tile([P, KT_ * D], FP8, name="wd_all")

    # Head-block column map: 0:64 = values, 64 = ones (denominator row),
    # 65:127 = filler ones feeding unread psum rows (deterministic, no
    # uninitialized weights entering the PE).
    for jp in range(4):
        nc.gpsimd.memset(vv[:, jp, :, :, DH:HB].bitcast(U8), ONE_FP8)

    # =========== phase 1: x load + wq/wk loads ===========
    # x: row-major load + PE transpose (evac on ScalarE, idle this early).
    # Weights: permuted DMA + DVE StreamTranspose + DVE fp8 cast; wv/wd are
    # emitted LATE so the early QK psum evacs aren't queued behind them in
    # the DVE stream.
    for st in range(ST_):
        nc.sync.dma_start(out=xn[st], in_=x_ap[ts(st, P), :])

    wsc_pool = ctx.enter_context(tc.tile_pool(name="wsc", bufs=1))
    _load_wT(nc, wq_all, wsc_pool, wq_ap, [nc.scalar, nc.sync], nc.vector,
             "wq")
    _load_wT(nc, wk_all, wsc_pool, wk_ap, [nc.scalar, nc.sync], nc.vector,
             "wk")

    # =========== phase 2: x transpose + QKV projections (DoubleRow) ========
    wqv = wq_all.rearrange("p (k f) -> p k f", f=D)
    wkv = wk_all.rearrange("p (k f) -> p k f", f=D)
    wvv = wv_all.rearrange("p (k f) -> p k f", f=D)

    with tc.tile_pool(name="ps_tv", bufs=2, space="PSUM") as psum_tv, \
         tc.tile_pool(name="ps_qk", bufs=2, space="PSUM") as psum_qk:

        # xT: evac on ScalarE (strided into x_all), DVE stays free
        for st in range(ST_):
            tps = psum_tv.tile([P, D], F32, tag="tps", bufs=2, name="tpsx")
            t3 = tps.rearrange("p (k b) -> p k b", b=P)
            for kt in range(KT_):
                nc.tensor.transpose(t3[:, kt], xn[st][:, ts(kt, P)], ident)
            nc.scalar.copy(xv[:, :, ds(st * P, P)], t3)

        def qk_proj(pr):
            for wv3, bias_t, dest in ((wqv, bq_t, qT), (wkv, bk_t, kT)):
                qps = psum_qk.tile([P, S], F32, tag="qkps", bufs=2,
                                   name="qps")
                for p2 in range(KT_ // 2):
                    for qc in range(0, S, 512):
                        nc.tensor.matmul(
                            qps[:, ds(qc, 512)],
                            lhsT=wv3[:, 2 * p2 : 2 * p2 + 2, ts(pr, P)],
                            rhs=xv[:, 2 * p2 : 2 * p2 + 2, ds(qc, 512)],
                            start=(p2 == 0),
                            stop=(p2 == KT_ // 2 - 1),
                            perf_mode=DR,
                        )
                if use_qkv_bias:
                    nc.vector.tensor_scalar_add(dest[pr], qps,
                                                bias_t[:, pr : pr + 1])
                else:
                    nc.vector.tensor_copy(dest[pr], qps)

        qk_proj(0)
        qk_proj(1)
        # wv load lands here in the DVE stream: after the first two QK evacs
        _load_wT(nc, wv_all, wsc_pool, wv_ap, [nc.gpsimd, nc.sync],
                 nc.vector, "wv")

        # V natural layout, per-head blocks in vpair
        for st in range(ST_):
            vps = psum_tv.tile([P, D], F32, tag="tps", bufs=2, name="vps")
            for p2 in range(KT_ // 2):
                for c0, cw in ((0, 512), (512, 256)):
                    nc.tensor.matmul(
                        vps[:, ds(c0, cw)],
                        lhsT=xv[:, 2 * p2 : 2 * p2 + 2, ts(st, P)],
                        rhs=wvv[:, 2 * p2 : 2 * p2 + 2, ds(c0, cw)],
                        start=(p2 == 0),
                        stop=(p2 == KT_ // 2 - 1),
                        perf_mode=DR,
                    )
            v3 = vps.rearrange("p (h c) -> p h c", c=DH)
            vdst = vv[:, st // 2, st % 2, :, 0:DH]
            if use_qkv_bias:
                stage = psum_tv.tile([P, D], F32, tag="tps", bufs=2,
                                     name="vstage")
                s3 = stage.rearrange("p (h c) -> p h c", c=DH)
                bv3 = bv_bc.rearrange("p (h c) -> p h c", c=DH)
                nc.vector.tensor_add(s3, v3, bv3)
                nc.vector.tensor_copy(vdst, s3)
            else:
                nc.vector.tensor_copy(vdst, v3)

        for pr in range(2, KT_):
            qk_proj(pr)

    # =========== phase 3: attention, two heads at a time ===========
    wdv = wd_all.rearrange("p (k f) -> p k f", f=D)
    with tc.tile_pool(name="expT", bufs=1) as exp_pool, \
         tc.tile_pool(name="den", bufs=1) as den_pool, \
         tc.tile_pool(name="ps_s", bufs=2, space="PSUM") as psum_s, \
         tc.tile_pool(name="ps_ctx", bufs=2, space="PSUM") as psum_ctx:

        def emit_ctx(pend):
            # ctx DoubleRow matmuls for one deferred (pair, jp) group
            pr, jp, cc, ets = pend
            for half in range(2):
                h = 2 * pr + half
                e3 = ets[half].rearrange("p (t q) -> p t q", q=S)
                for qc in range(0, S, 512):
                    nc.tensor.matmul(
                        cc[half][:, ds(qc, 512)],
                        lhsT=vv[:, jp, :, h, :],
                        rhs=e3[:, :, ds(qc, 512)],
                        start=(jp == 0),
                        stop=(jp == 3),
                        perf_mode=DR,
                    )
            if jp == 3:
                emit_den(pr, cc)

        def emit_den(pr, cc):
            # normalize: ctxT = cc[0:64] / cc[64] into ctx_all (fp8)
            for half in range(2):
                h = 2 * pr + half
                kt = h // 2
                den_sb = den_pool.tile([1, S], F32, tag="den_sb", bufs=2)
                nc.vector.tensor_copy(den_sb, cc[half][DH : DH + 1, :])
                rec = den_pool.tile([1, S], F32, tag="rec", bufs=2)
                nc.vector.reciprocal_approx_fast(rec, den_sb)
                recb = den_pool.tile([DH, S], F32, tag="recb", bufs=2)
                nc.gpsimd.partition_broadcast(recb, rec)
                nc.vector.tensor_mul(
                    ctx_all[DH * (h % 2) : DH * (h % 2) + DH, ts(kt, S)],
                    cc[half][0:DH, :], recb)

        pending = None  # deferred ctx group: hides under the next jp's exps
        for pr in range(H // 2):
            if pr == 1:
                # overlap the Wd transpose-load with attention compute
                _load_wT(nc, wd_all, wsc_pool, wd_ap, [nc.gpsimd, nc.sync],
                         nc.vector, "wd")
            cc = [psum_ctx.tile([HB, S], F32, tag="cps", bufs=2,
                                name=f"cps{half}") for half in range(2)]
            et = [None, None]
            for j in range(ST_):
                jp, jh = j // 2, j % 2
                for half in range(2):
                    hp = DH * half
                    if jh == 0:
                        et[half] = exp_pool.tile([P, 2 * S], FP8,
                                                 tag=f"e{half}", bufs=2,
                                                 name=f"e{half}")
                    sps = psum_s.tile([P, S], F32, tag="sps", bufs=2,
                                      name=f"sps{half}")
                    for qc in range(0, S, 512):
                        nc.tensor.matmul(
                            sps[:, ds(qc, 512)],
                            lhsT=kT[pr][hp : hp + DH, ts(j, P)],
                            rhs=qT[pr][hp : hp + DH, ds(qc, 512)],
                            start=True,
                            stop=True,
                        )
                    nc.scalar.activation(
                        et[half][:, ds(jh * S, S)], sps, FT.Exp,
                        bias=(maskT[:, j : j + 1] if use_mask else 0.0),
                        scale=0.125,
                    )
                if jh == 1:
                    if pending is not None:
                        emit_ctx(pending)
                    pending = (pr, jp, cc, (et[0], et[1]))
        emit_ctx(pending)

    # =========== phase 4: dense + residual + layernorm ===========
    with tc.tile_pool(name="ln", bufs=2) as ln_pool, \
         tc.tile_pool(name="stat", bufs=4) as stat_pool, \
         tc.tile_pool(name="osb", bufs=3) as out_pool, \
         tc.tile_pool(name="ps_o", bufs=2, space="PSUM") as psum_o:

        for st in range(ST_):
            xr = xn[st]
            if use_dense_bias:
                xb = ln_pool.tile([P, D], F32, tag="xb")
                nc.vector.tensor_add(xb, xr, bd_bc)
                xr = xb
            ops = psum_o.tile([P, D], F32, tag="ops", bufs=2)
            for p2 in range(KT_ // 2):
                for c0, cw in ((0, 512), (512, 256)):
                    nc.tensor.matmul(
                        ops[:, ds(c0, cw)],
                        lhsT=cxv[:, 2 * p2 : 2 * p2 + 2, ts(st, P)],
                        rhs=wdv[:, 2 * p2 : 2 * p2 + 2, ds(c0, cw)],
                        start=(p2 == 0),
                        stop=(p2 == KT_ // 2 - 1),
                        perf_mode=DR,
                    )
            # full = dense_out + x, accumulating the row-sum on the fly
            full = ln_pool.tile([P, D], F32, tag="full")
            sums = stat_pool.tile([P, 1], F32, tag="sums")
            nc.vector.scalar_tensor_tensor(
                out=full, in0=ops, scalar=1.0, in1=xr,
                op0=ALU.mult, op1=ALU.add, accum_out=sums,
            )
            # sum of squares on ScalarE; sq is a dead store
            sq = ln_pool.tile([P, D], F32, tag="sq")
            ssq = stat_pool.tile([P, 1], F32, tag="ssq")
            nc.scalar.activation(sq, full, FT.Square, accum_out=ssq)
            mu = stat_pool.tile([P, 1], F32, tag="mu")
            nc.vector.tensor_scalar_mul(mu, sums, 1.0 / D)
            mu2 = stat_pool.tile([P, 1], F32, tag="mu2")
            nc.vector.tensor_scalar_mul(mu2, mu, mu)
            var = stat_pool.tile([P, 1], F32, tag="var")
            nc.vector.scalar_tensor_tensor(
                out=var, in0=ssq, scalar=1.0 / D, in1=mu2,
                op0=ALU.mult, op1=ALU.subtract,
            )
            std = stat_pool.tile([P, 1], F32, tag="std")
            nc.scalar.activation(std, var, FT.Sqrt, bias=eps_t)
            rstd = stat_pool.tile([P, 1], F32, tag="rstd")
            nc.vector.reciprocal(rstd, std)
            osb = out_pool.tile([P, D], F32, tag="osb")
            nc.vector.tensor_scalar(
                out=osb, in0=full, scalar1=mu, scalar2=rstd,
                op0=ALU.subtract, op1=ALU.mult,
            )
            if use_ln_affine:
                nc.vector.tensor_mul(osb, osb, g_bc)
                nc.vector.tensor_add(osb, osb, b_bc)
            nc.sync.dma_start(out=out_ap[ts(st, P), :], in_=osb)


def build(flags):
    nc = bacc.Bacc(
        "TRN2", target_bir_lowering=False, debug=False, num_devices=N_CORES
    )
    aps = {}
    for name, shape in (
        ("hidden_states", [S, D]),
        ("attention_mask", [S]),
        ("Wq", [D, D]), ("bq", [D]),
        ("Wk", [D, D]), ("bk", [D]),
        ("Wv", [D, D]), ("bv", [D]),
        ("Wd", [D, D]), ("bd", [D]),
        ("ln_g", [D]), ("ln_b", [D]),
    ):
        aps[name] = nc.dram_tensor(name, shape, F32, kind="ExternalInput").ap()
    out = nc.dram_tensor("out", [S, D], F32, kind="ExternalOutput").ap()

    with tile.TileContext(nc) as tc:
        bert_attn_kernel(
            tc, out,
            aps["hidden_states"], aps["attention_mask"],
            aps["Wq"], aps["bq"], aps["Wk"], aps["bk"],
            aps["Wv"], aps["bv"], aps["Wd"], aps["bd"],
            aps["ln_g"], aps["ln_b"],
            *flags,
        )
    nc.compile()
    return nc


_CACHE = {}
last_results = None  # BassKernelResults of the most recent run (for test.py)


def kernel(**inputs):
    xs = {k: np.ascontiguousarray(np.asarray(v, dtype=np.float32))
          for k, v in inputs.items()}
    B = xs["hidden_states"].shape[0]
    assert B == N_CORES

    flags = (
        bool(np.any(xs["attention_mask"])),
        bool(np.any(xs["bq"]) or np.any(xs["bk"]) or np.any(xs["bv"])),
        bool(np.any(xs["bd"])),
        bool(np.any(xs["ln_g"] != 1.0) or np.any(xs["ln_b"])),
    )
    if flags not in _CACHE:
        _CACHE[flags] = build(flags)
    nc = _CACHE[flags]

    shared = {k: xs[k] for k in
              ("Wq", "bq", "Wk", "bk", "Wv", "bv", "Wd", "bd", "ln_g", "ln_b")}
    in_maps = [
        dict(
            hidden_states=xs["hidden_states"][i],
            attention_mask=np.ascontiguousarray(
                xs["attention_mask"][i].reshape(S)),
            **shared,
        )
        for i in range(N_CORES)
    ]
    trace = bool(int(os.environ.get("BERT_KERNEL_TRACE", "0")))
    res = run_bass_kernel_spmd(
        nc, in_maps, core_ids=list(range(N_CORES)), trace=trace
    )
    global last_results
    last_results = res
    return np.stack([res.results[i]["out"] for i in range(N_CORES)], axis=0)


if __name__ == "__main__":
    rng = np.random.default_rng(0)
    ins = {
        "hidden_states": rng.standard_normal((8, S, D), dtype=np.float32),
        "attention_mask": np.zeros((8, 1, 1, S), np.float32),
        "Wq": rng.standard_normal((D, D), dtype=np.float32) * 0.02,
        "bq": np.zeros(D, np.float32),
        "Wk": rng.standard_normal((D, D), dtype=np.float32) * 0.02,
        "bk": np.zeros(D, np.float32),
        "Wv": rng.standard_normal((D, D), dtype=np.float32) * 0.02,
        "bv": np.zeros(D, np.float32),
        "Wd": rng.standard_normal((D, D), dtype=np.float32) * 0.02,
        "bd": np.zeros(D, np.float32),
        "ln_g": np.ones(D, np.float32),
        "ln_b": np.zeros(D, np.float32),
    }
    out = kernel(**ins)
    print(out.shape, out.dtype, np.abs(out).max())
